# revision 51
# baseline (speedup 1.0000x reference)
"""Trainium2 Bass kernel: GQA sliding-window attention (v2, fp16).

Problem: B=1, T=4096, D=2048, H=16 q-heads, KVH=4 kv-heads, HD=128,
causal sliding window 512.

Sharding: 8-way sequence parallel. Core c owns query rows
[512c, 512c+512). It receives x rows [512(c-1), 512(c+1)) (halo of 512
rows; core 0's halo is zeros). Weights replicated. Outputs are disjoint
row blocks -> plain concatenation, no collectives.

v2 changes vs baseline:
  - fp16 everywhere on-chip (f32 PSUM accumulation); inputs cast and
    pre-laid-out host-side (x pre-transposed -> no P0 transpose phase,
    weights in exact SBUF layouts -> linear DMA).
  - scores: 512+128-col matmuls per (head, q-tile); additive edge masks
    (two [128,128] f32 triangles built on-chip via affine_select)
    applied to the two edge 128-col blocks of the PSUM band in ONE
    strided DVE add; middle 384 cols never need masking.
  - exp on ACT with scale=SCALE folded in and accum_out giving the
    masked row sums directly; per-row 1/l folded into the PE transpose
    of the weights (regular matmul with rhs = diag(r)).
  - banded PV: per q-tile only its 5 key chunks accumulate -> no wT
    zero-fill.
  - core 0's zero-halo keys contribute exp(0)=1 each to the row sums;
    corrected via the lcorr input (-count per row, zeros on cores>0).
  - 3-stage pipelined head loop: iteration h emits scores/exp of head
    h, q-proj of h+1, transposes of h-1, PV of h-2 (wT double-buffered
    by head parity) so every cross-engine chain has PE filler.
  - drain interleaves the first Wo block's partial accumulations; P3
    streams the rest from the freed score PSUM banks.

Per-core layouts (SBUF partition dim first, all fp16 except f32 sums):
  xt  [128, 16, 1024] : xt[p, dc, j] = x_c[j, 128*dc+p]   (host prep)
  qT  [128, 16, 512]  : qT[p, h, i]  = q[i, 128*h+p]  (unscaled)
  kT  [128, 4, 1024]  : kT[p, g, j]  = k[j, 128*g+p]
  vv  [128, 8, 512]   : vv[p, jc, e] = v[128*jc+p, e]
  oT  [128, 16, 512]  : oT[p, h, i]  = attn_out[i, 128*h+p]
  y = oT.T @ Wo accumulated over heads, streamed in 512-col blocks,
  written fp16 and upcast host-side.
"""

import numpy as np

T = 4096
D = 2048
H = 16
KVH = 4
HD = 128
WINDOW = 512
SCALE = HD ** -0.5
N_CORES = 8
TLOC = T // N_CORES          # 512 own query rows / core
XROWS = TLOC + WINDOW        # 1024 x rows / core (halo + own)
NT = TLOC // 128             # 4 q-tiles of 128 rows
NJC = XROWS // 128           # 8 key chunks of 128
BAND = WINDOW + 128          # 640 key columns per q-tile
DC = D // 128                # 16 d-chunks
MASK_VAL = -1e9

_CACHE = {}


def _emit(nc, tc, tile, mybir, make_identity, loop_n=None, stop_after=None):
    f32 = mybir.dt.float32
    f16 = mybir.dt.float16

    timing = loop_n is not None
    kin = "Internal" if timing else "ExternalInput"
    kout = "Internal" if timing else "ExternalOutput"
    xt_d = nc.dram_tensor("xt", [128, DC * XROWS], f16, kind=kin)
    wq_d = nc.dram_tensor("wq", [128, H * DC * 128], f16, kind=kin)
    wk_d = nc.dram_tensor("wk", [128, KVH * DC * 128], f16, kind=kin)
    wv_d = nc.dram_tensor("wv", [128, DC * 512], f16, kind=kin)
    wo_d = nc.dram_tensor("wo", [128, 4 * H * 512], f16, kind=kin)
    lcorr_d = nc.dram_tensor("lcorr", [128, NT], f32, kind=kin)
    y_d = nc.dram_tensor("y", [TLOC, D], f16, kind=kout)
    if timing:
        dummy_d = nc.dram_tensor("bench_done", [1, 128], f32,
                                 kind="ExternalOutput")

    def mm(out, lhsT, rhs, start, stop):
        nc.tensor.matmul(out, lhsT, rhs, start=start, stop=stop)

    # --- long-lived pools / loop-invariant tiles ---
    # PSUM budget (8 banks): ps_s 2 bufs x [128,640]f32 = 4 banks,
    # ps_ot 2 bufs x [128,512]f32 = 2 banks, plus one phase-scoped
    # right-side pool of <=2 banks (ps_acc in P1, ps_wtp in P2,
    # ps_acc2 in P3).
    pers = tc.alloc_tile_pool(name="pers", bufs=1)
    ps_s = tc.alloc_tile_pool(name="ps_s", bufs=2, space="PSUM")
    ps_ot = tc.alloc_tile_pool(name="ps_ot", bufs=2, space="PSUM")

    ident = pers.tile([128, 128], f16, tag="ident")
    make_identity(nc, ident[:])
    # additive edge masks, stacked [128, 2, 128]: block 0 = maskL for
    # band cols [0,128) (allowed jj >= p), block 1 = maskR for cols
    # [512,640) (allowed jj-512 <= p). Applied as ONE strided DVE add.
    masks = pers.tile([128, 2, 128], f32, tag="masks")
    nc.gpsimd.memset(masks[:], 0.0)
    nc.gpsimd.affine_select(
        out=masks[:, 0, :], in_=masks[:, 0, :],
        compare_op=mybir.AluOpType.is_ge,
        fill=MASK_VAL, base=0, pattern=[[1, 128]], channel_multiplier=-1)
    nc.gpsimd.affine_select(
        out=masks[:, 1, :], in_=masks[:, 1, :],
        compare_op=mybir.AluOpType.is_ge,
        fill=MASK_VAL, base=0, pattern=[[-1, 128]], channel_multiplier=1)

    lp = tc.For_i(0, loop_n, 1) if timing else None
    if lp is not None:
        lp.__enter__()

    proj = tc.alloc_tile_pool(name="proj", bufs=1)
    xp = tc.alloc_tile_pool(name="xp", bufs=1)
    wp = tc.alloc_tile_pool(name="wpool", bufs=2)
    ps_acc = tc.alloc_tile_pool(name="ps_acc", bufs=2, space="PSUM",
                                side="right")

    qT = proj.tile([128, H, TLOC], f16, tag="qT")
    kT = proj.tile([128, KVH, XROWS], f16, tag="kT")
    vv = proj.tile([128, NJC, KVH * HD], f16, tag="vv")
    lcorr_s = proj.tile([128, NT], f32, tag="lcorr")
    xt = xp.tile([128, DC, XROWS], f16, tag="xt")

    nc.sync.dma_start(lcorr_s[:], lcorr_d.ap())

    # ---------------- P1a: k projections (x streamed in) -------------
    # The prologue is DMA-bound (serial transfer resource), so k-proj
    # runs dc-OUTER with all 8 (kv-head, half) accumulation groups open
    # at once across all 8 PSUM banks: each arriving x chunk is fully
    # consumed (8 x 512-col matmuls) before the next chunk lands.
    # DMA order = consumption order: wk g0/g1, x chunks, wk g2/g3, wv.
    wkgs = []
    for g in range(KVH):
        wkg = wp.tile([128, DC, 128], f16, tag="wlhs", name=f"wkg{g}",
                      bufs=4)
        wkgs.append(wkg)
    # first dc-slice of wk g0 split out so the very first matmul can
    # start ~1.5us earlier on the serial DMA stream
    nc.sync.dma_start(wkgs[0][:, 0, :], wk_d.ap()[:, 0:128])
    nc.sync.dma_start(xt[:, 0, :], xt_d.ap()[:, 0:XROWS])
    nc.sync.dma_start(wkgs[0][:, 1:DC, :], wk_d.ap()[:, 128:DC * 128])
    for dc in range(1, DC):
        nc.sync.dma_start(xt[:, dc, :],
                          xt_d.ap()[:, dc * XROWS:(dc + 1) * XROWS])
        if dc == 1:
            # g1 weights after the first x chunks: g0's matmuls cover
            # the PE meanwhile
            nc.sync.dma_start(wkgs[1][:],
                              wk_d.ap()[:, DC * 128:2 * DC * 128])
    nc.sync.dma_start(wkgs[2][:], wk_d.ap()[:, 2 * DC * 128:3 * DC * 128])
    nc.sync.dma_start(wkgs[3][:], wk_d.ap()[:, 3 * DC * 128:4 * DC * 128])

    def kproj_pair(g0, g1, slots, warmup=0):
        """dc-outer over two kv heads: 4 open accumulation groups;
        each x chunk fully consumed on arrival (4 x 512-col matmuls ~
        one chunk's DMA time). warmup: emit g0's first `warmup` chunks
        before g1's so the in-order PE queue isn't blocked on g1's
        weight DMA at startup."""
        gs = [g0, g0, g1, g1]
        for dc in range(warmup):
            for s in range(2):
                mm(slots[s], wkgs[gs[s]][:, dc, :],
                   xt[:, dc, (s % 2) * 512:(s % 2 + 1) * 512],
                   start=(dc == 0), stop=(dc == DC - 1))
        for dc in range(warmup):
            for s in range(2, 4):
                mm(slots[s], wkgs[gs[s]][:, dc, :],
                   xt[:, dc, (s % 2) * 512:(s % 2 + 1) * 512],
                   start=(dc == 0), stop=(dc == DC - 1))
        for dc in range(warmup, DC):
            for s in range(4):
                mm(slots[s], wkgs[gs[s]][:, dc, :],
                   xt[:, dc, (s % 2) * 512:(s % 2 + 1) * 512],
                   start=(dc == 0), stop=(dc == DC - 1))
        for s in range(4):
            # GPSIMD cannot access PSUM on HW: copies go DVE/ACT only
            dst = kT[:, gs[s], (s % 2) * 512:(s % 2 + 1) * 512]
            if s % 2 == 0:
                nc.vector.tensor_copy(dst, slots[s])
            else:
                nc.scalar.copy(dst, slots[s])

    # phase A in 2x[128,1024] ps_s tiles; phase B in ps_ot/ps_acc slots
    # so it does not wait on phase A's PSUM->SBUF copies.
    pkA = [ps_s.tile([128, 1024], f32, tag="score", name=f"pkA{i}")
           for i in range(2)]
    kproj_pair(0, 1, [pkA[0][:, 0:512], pkA[0][:, 512:1024],
                      pkA[1][:, 0:512], pkA[1][:, 512:1024]], warmup=3)
    pkO = [ps_ot.tile([128, TLOC], f32, tag="ot", name=f"pkO{i}")
           for i in range(2)]
    pkB = [ps_acc.tile([128, 512], f32, tag="acc", name=f"pkB{i}")
           for i in range(2)]
    kproj_pair(2, 3, [pkO[0][:], pkO[1][:], pkB[0][:], pkB[1][:]])

    # ---------------- P1b: v projections ------------------------------
    # chunks 6 and 7 are deferred into head-0's iteration as PE filler
    # (the 3-stage pipeline has no transpose/PV work for h=0 yet).
    wvt = wp.tile([128, DC, 512], f16, tag="wv", name="wvt", bufs=1)
    nc.sync.dma_start(wvt[:], wv_d.ap())
    for jc in range(NJC - 3):
        pv = ps_acc.tile([128, 512], f32, tag="acc")
        for dc in range(DC):
            mm(pv[:], xt[:, dc, jc * 128:(jc + 1) * 128], wvt[:, dc, :],
               start=(dc == 0), stop=(dc == DC - 1))
        nc.vector.tensor_copy(vv[:, jc, :], pv[:])

    ps_acc.release()

    if stop_after == "kv":
        if lp is not None:
            lp.__exit__(None, None, None)
            dtile = pers.tile([128, 128], f32, tag="dtile")
            nc.vector.memset(dtile[:], 0.0)
            nc.sync.dma_start(dummy_d.ap(), dtile[0:1, :])
        wp.release()
        xp.release()
        proj.release()
        ps_ot.release()
        ps_s.release()
        pers.release()
        return

    # ---------------- P2: attention, q projection interleaved ---------
    attn = tc.alloc_tile_pool(name="attn", bufs=1, side="right")
    # Wo stream pool allocated before sm/ps_wtp (right-side pools are
    # released in stack order) so the first two 2 MiB chunks can be
    # prefetched during the attention tail.
    wop = tc.alloc_tile_pool(name="wo_pool", bufs=2, side="right")
    sm = tc.alloc_tile_pool(name="sm", bufs=2, side="right")
    ps_wtp = tc.alloc_tile_pool(name="ps_wtp", bufs=2, space="PSUM",
                                side="right")

    oT = attn.tile([128, H, TLOC], f16, tag="oT")
    # wT double-buffered by head parity: transposes of head h write
    # wTs[h%2] while PV of head h-1 still reads wTs[(h-1)%2].
    wTs = [attn.tile([128, NJC, TLOC], f16, tag=f"wT{i}", name=f"wT{i}")
           for i in range(2)]

    woc_tiles = {}

    def wo_load(dblk):
        woc = wop.tile([128, H, 512], f16, tag="wo", name=f"wo{dblk}")
        nc.sync.dma_start(
            woc[:], wo_d.ap()[:, dblk * H * 512:(dblk + 1) * H * 512])
        woc_tiles[dblk] = woc

    # --- per-head emission pieces (closures over head state) ---------

    def make_scores(h):
        """Returns (score(t) emitters, state). score(t): 2 matmuls +
        one strided DVE edge-mask add + ACT exp with accum row sums."""
        g = h // (H // KVH)
        wt_list = []
        lsum4 = sm.tile([128, NT], f32, tag="l4", name=f"l4_{h}", bufs=2)
        lfin4 = sm.tile([128, NT], f32, tag="lf4", name=f"lf4_{h}", bufs=2)
        r4 = sm.tile([128, NT], f32, tag="r4", name=f"r4_{h}", bufs=2)
        rd = sm.tile([128, NT, 128], f16, tag="rd", name=f"rd_{h}", bufs=2)

        def score(t):
            # [128,1024] = 2 full PSUM banks; matmul outputs must not
            # cross a bank boundary, so the 640-col band is written as
            # 512 + 128 cols. DVE/ACT read the 640 contiguously.
            ps = ps_s.tile([128, 1024], f32, tag="score", name=f"s{h}_{t}")
            mm(ps[:, 0:512], qT[:, h, t * 128:(t + 1) * 128],
               kT[:, g, t * 128:t * 128 + 512], start=True, stop=True)
            mm(ps[:, 512:BAND], qT[:, h, t * 128:(t + 1) * 128],
               kT[:, g, t * 128 + 512:t * 128 + BAND], start=True, stop=True)
            # both edge-mask adds in one strided DVE op: blocks at
            # cols [0,128) and [512,640)
            ps_edges = ps[:, 0:1024].rearrange(
                "p (a b) -> p a b", a=2)[:, :, 0:128]
            nc.vector.tensor_add(ps_edges, ps_edges, masks[:])
            w = sm.tile([128, BAND], f16, tag="w", name=f"w{h}_{t}", bufs=11)
            nc.scalar.activation(w[:], ps[:, 0:BAND],
                                 mybir.ActivationFunctionType.Exp,
                                 scale=SCALE,
                                 accum_out=lsum4[:, t:t + 1])
            wt_list.append(w)

        def fin():
            """reduction chain; emitted late so lfin/recip (gated on
            exp t3) don't head-of-line block wT copies on DVE."""
            nc.vector.tensor_add(lfin4[:], lsum4[:], lcorr_s[:])
            nc.vector.reciprocal(r4[:], lfin4[:])
            for t in range(NT):
                # diag(r_t) built on the (idle) Pool engine
                nc.gpsimd.tensor_scalar_mul(rd[:, t, :], ident[:],
                                            r4[:, t:t + 1])

        return score, fin, (wt_list, rd, h)

    def make_qp(hq):
        """q-projection matmul chunks for head hq (PE filler)."""
        if hq >= H:
            return (lambda lo, hi: None), (lambda: None)
        wqh = wp.tile([128, DC, 128], f16, tag="wlhs", name=f"wqh{hq}",
                      bufs=4)
        nc.sync.dma_start(
            wqh[:], wq_d.ap()[:, hq * DC * 128:(hq + 1) * DC * 128])
        pq = ps_ot.tile([128, TLOC], f32, tag="ot", name=f"pq{hq}")

        def qp_mms(dc_lo, dc_hi):
            for dc in range(dc_lo, dc_hi):
                mm(pq[:], wqh[:, dc, :], xt[:, dc, WINDOW:XROWS],
                   start=(dc == 0), stop=(dc == DC - 1))

        def qp_fin():
            # (GPSIMD cannot access PSUM on HW)
            nc.scalar.copy(qT[:, hq, :], pq[:])

        return qp_mms, qp_fin

    def make_tgroups(state):
        """transpose(+1/l)/copy emitters for head h's weights into
        wTs[h%2]."""
        wt_list, rd, h = state
        wT = wTs[h % 2]

        def tgroup(jc):
            t_lo = max(0, jc - 4)
            t_hi = min(NT - 1, jc)
            pt = ps_wtp.tile([128, 512], f32, tag="wtp", name=f"pt{h}_{jc}")
            for t in range(t_lo, t_hi + 1):
                co = jc - t  # w column block
                # transpose-with-normalization as a regular matmul:
                # pt[:, t] = w_t_co.T @ diag(r_t)  (is_transpose only
                # allows permutation rhs, so use the plain PE path)
                mm(pt[:, t * 128:(t + 1) * 128],
                   wt_list[t][:, co * 128:(co + 1) * 128],
                   rd[:, t, :], start=True, stop=True)
            if jc % 2 == 0:
                nc.vector.tensor_copy(
                    wT[:, jc, t_lo * 128:(t_hi + 1) * 128],
                    pt[:, t_lo * 128:(t_hi + 1) * 128])
            else:
                nc.scalar.copy(
                    wT[:, jc, t_lo * 128:(t_hi + 1) * 128],
                    pt[:, t_lo * 128:(t_hi + 1) * 128])

        return tgroup

    def make_pv(state):
        """banded PV emitters for head h, reading wTs[h%2] (whose copies
        completed a full iteration earlier) + the final oT copy."""
        _, _, h = state
        g = h // (H // KVH)
        wT = wTs[h % 2]
        po = ps_ot.tile([128, TLOC], f32, tag="ot", name=f"po{h}")

        def pv(t):
            # one accumulation group open at a time per PSUM bank
            # (a start marks the whole 2KB zero region pending).
            for i, jc in enumerate(range(t, t + 5)):
                mm(po[:, t * 128:(t + 1) * 128],
                   vv[:, jc, g * 128:(g + 1) * 128],
                   wT[:, jc, t * 128:(t + 1) * 128],
                   start=(i == 0), stop=(i == 4))

        def ot_fin():
            # (GPSIMD cannot access PSUM on HW)
            nc.vector.tensor_copy(oT[:, h, :], po[:])

        return pv, ot_fin

    # --- 3-stage pipelined head loop ---------------------------------
    # Iteration h emits: scores/exp of head h, q-proj of h+1,
    # transposes of h-1, PV of h-2 — interleaved so every cross-engine
    # consumer chain has PE filler in front of it.

    # head 0's q-projection runs standalone (heads h+1 ride iteration h)
    qp0_mms, qp0_fin = make_qp(0)
    qp0_mms(0, DC)
    qp0_fin()

    tg_state = None   # (wt_list, rd, h) for head h-1 (transpose stage)
    pv_state = None   # same for head h-2 (PV stage)
    for h in range(H):
        score, fin, cur_state = make_scores(h)
        qp_mms, qp_fin = make_qp(h + 1)
        tgroup = make_tgroups(tg_state) if tg_state is not None else None
        if pv_state is not None:
            pv, ot_fin = make_pv(pv_state)
        else:
            pv = ot_fin = None

        def T(jc):
            if tgroup is not None:
                tgroup(jc)

        def PV(t):
            if pv is not None:
                pv(t)

        def vv_late(jc, eng):
            """deferred v-projection chunk (PE filler in iteration 0)"""
            pvv = ps_ot.tile([128, TLOC], f32, tag="ot", name=f"pvv{jc}")
            for dc in range(DC):
                mm(pvv[:], xt[:, dc, jc * 128:(jc + 1) * 128],
                   wvt[:, dc, :], start=(dc == 0), stop=(dc == DC - 1))
            if eng == 0:
                nc.vector.tensor_copy(vv[:, jc, :], pvv[:])
            else:
                nc.scalar.copy(vv[:, jc, :], pvv[:])

        score(0)
        T(0)
        score(1)
        T(1)
        qp_mms(0, 4)
        score(2)
        T(2)
        qp_mms(4, 8)
        score(3)
        T(3)
        qp_mms(8, DC)
        if h == 0:
            vv_late(6, 0)
        elif h == 1:
            vv_late(5, 0)
        T(4)
        PV(0)
        T(5)
        PV(1)
        T(6)
        PV(2)
        T(7)
        PV(3)
        if h == 0:
            vv_late(7, 1)
        if ot_fin is not None:
            ot_fin()
        if h == 12:
            wo_load(0)
        elif h == 14:
            wo_load(1)
        fin()
        qp_fin()
        pv_state = tg_state
        tg_state = cur_state

    # --- drain: transposes of head 15, PV of heads 14 and 15, with the
    # first Wo block's partial accumulations (heads 0..13) as PE filler.
    tgroup15 = make_tgroups(tg_state)
    pv14, ot_fin14 = make_pv(pv_state)

    py_tiles = {}  # t -> (tile, col half)

    def py_open(dblk):
        pyA = ps_s.tile([128, 1024], f32, tag="score", name=f"pyA{dblk}")
        pyB = ps_s.tile([128, 1024], f32, tag="score", name=f"pyB{dblk}")
        for t in range(NT):
            py_tiles[t] = (pyA if t < 2 else pyB, t % 2)

    def py_mms(t, h_lo, h_hi, dblk):
        woc = woc_tiles[dblk]
        py, half = py_tiles[t]
        for h2 in range(h_lo, h_hi):
            mm(py[:, half * 512:(half + 1) * 512],
               oT[:, h2, t * 128:(t + 1) * 128], woc[:, h2, :],
               start=(h2 == 0), stop=(h2 == H - 1))

    skip_p3 = stop_after == "attn"
    if not skip_p3:
        py_open(0)

    def PY(t, h_lo, h_hi):
        if not skip_p3:
            py_mms(t, h_lo, h_hi, 0)

    tgroup15(0)
    tgroup15(1)
    PY(0, 0, 7)
    tgroup15(2)
    pv14(0)
    PY(0, 7, 14)
    tgroup15(3)
    pv14(1)
    PY(1, 0, 7)
    tgroup15(4)
    pv14(2)
    PY(1, 7, 14)
    tgroup15(5)
    pv14(3)
    tgroup15(6)
    tgroup15(7)
    ot_fin14()

    pv15, ot_fin15 = make_pv(tg_state)
    pv15(0)
    PY(2, 0, 7)
    pv15(1)
    PY(2, 7, 14)
    pv15(2)
    PY(3, 0, 7)
    pv15(3)
    PY(3, 7, 14)
    ot_fin15()

    sm.release()
    ps_wtp.release()
    wp.release()
    xp.release()
    proj.release()

    if stop_after == "attn":
        if lp is not None:
            lp.__exit__(None, None, None)
            dtile = pers.tile([128, 128], f32, tag="dtile")
            nc.vector.memset(dtile[:], 0.0)
            nc.sync.dma_start(dummy_d.ap(), dtile[0:1, :])
        wop.release()
        attn.release()
        ps_ot.release()
        ps_s.release()
        pers.release()
        return

    # ---------------- P3: output projection ----------------
    # dblk 0's heads 0..13 already accumulated during the drain above;
    # finish with heads 14/15, then stream the remaining Wo blocks.
    # py tiles come from the ps_s pool (tag "score") — its banks are
    # free once the last exps have consumed the final score tiles.
    def y_out(t, dblk, py_ap):
        ych = attn.tile([128, 512], f16, tag="ych", bufs=3)
        nc.vector.tensor_copy(ych[:], py_ap)
        nc.scalar.dma_start(
            y_d.ap()[t * 128:(t + 1) * 128,
                     dblk * 512:(dblk + 1) * 512],
            ych[:])

    # all h=14 contributions first (oT14 ready early), so the PE has
    # work while the Pool copy of oT15 drains
    for t in range(NT):
        py_mms(t, H - 2, H - 1, 0)
    for t in range(NT):
        py_mms(t, H - 1, H, 0)
        py, half = py_tiles[t]
        y_out(t, 0, py[:, half * 512:(half + 1) * 512])

    for dblk in range(1, 4):
        if dblk not in woc_tiles:
            wo_load(dblk)
        woc = woc_tiles[dblk]
        for t in range(NT):
            # alternate PSUM pools between dblks so a block's first
            # matmuls never wait on the previous block's output copies
            if dblk % 2 == 0:
                py = ps_s.tile([128, 1024], f32, tag="score",
                               name=f"py{dblk}_{t}")
                py_ap = py[:, 0:512]
            else:
                py = ps_ot.tile([128, TLOC], f32, tag="ot",
                                name=f"py{dblk}_{t}")
                py_ap = py[:]
            for h in range(H):
                mm(py_ap, oT[:, h, t * 128:(t + 1) * 128],
                   woc[:, h, :], start=(h == 0), stop=(h == H - 1))
            y_out(t, dblk, py_ap)

    wop.release()
    attn.release()

    if lp is not None:
        lp.__exit__(None, None, None)
        dtile = pers.tile([128, 128], f32, tag="dtile")
        nc.vector.memset(dtile[:], 0.0)
        nc.sync.dma_start(dummy_d.ap(), dtile[0:1, :])

    ps_ot.release()
    ps_s.release()
    pers.release()


def build_nc(loop_n=None, stop_after=None):
    key = ("nc", loop_n, stop_after)
    if key in _CACHE:
        return _CACHE[key]
    import concourse.bacc as bacc
    import concourse.mybir as mybir
    import concourse.tile as tile
    from concourse.masks import make_identity

    nc = bacc.Bacc("TRN2", target_bir_lowering=False, debug=False,
                   num_devices=N_CORES)
    with tile.TileContext(nc) as tc:
        _emit(nc, tc, tile, mybir, make_identity, loop_n=loop_n,
              stop_after=stop_after)
    nc.compile()
    _CACHE[key] = nc
    return nc


def make_inputs_for_core(c, xf, Wq, Wk, Wv, Wo):
    """xf: [T, D] float32 (already squeezed)."""
    f16 = np.float16
    if c == 0:
        x_c = np.concatenate(
            [np.zeros((WINDOW, D), np.float32), xf[:TLOC]], axis=0)
    else:
        x_c = xf[TLOC * c - WINDOW: TLOC * c + TLOC]

    # xt[p, dc, j] = x_c[j, 128*dc+p]
    xt = np.ascontiguousarray(
        x_c.reshape(XROWS, DC, 128).transpose(2, 1, 0).astype(f16)
    ).reshape(128, DC * XROWS)
    # wq[p, h, dc, e] = Wq[128*dc+p, 128*h+e]
    wq = np.ascontiguousarray(
        Wq.reshape(DC, 128, H, 128).transpose(1, 2, 0, 3).astype(f16)
    ).reshape(128, H * DC * 128)
    # wk[p, g, dc, e] = Wk[128*dc+p, 128*g+e]
    wk = np.ascontiguousarray(
        Wk.reshape(DC, 128, KVH, 128).transpose(1, 2, 0, 3).astype(f16)
    ).reshape(128, KVH * DC * 128)
    # wv[p, dc, e] = Wv[128*dc+p, e]
    wv = np.ascontiguousarray(
        Wv.reshape(DC, 128, KVH * HD).transpose(1, 0, 2).astype(f16)
    ).reshape(128, DC * 512)
    # wo[p, dblk, h, e] = Wo[128*h+p, 512*dblk+e]
    wo = np.ascontiguousarray(
        Wo.reshape(H, 128, 4, 512).transpose(1, 2, 0, 3).astype(f16)
    ).reshape(128, 4 * H * 512)

    # core 0: rows see (512 - i) spurious zero-halo keys, each exp(0)=1
    lcorr = np.zeros((128, NT), np.float32)
    if c == 0:
        p = np.arange(128)[:, None]
        t = np.arange(NT)[None, :]
        lcorr = -np.maximum(0, (512 - 128 * t) - p).astype(np.float32)

    return {
        "xt": xt,
        "wq": wq,
        "wk": wk,
        "wv": wv,
        "wo": wo,
        "lcorr": np.ascontiguousarray(lcorr),
    }


def kernel(x, Wq, Wk, Wv, Wo):
    from concourse.bass_utils import run_bass_kernel_spmd

    nc = build_nc()
    xf = np.asarray(x, np.float32).reshape(T, D)
    Wq = np.asarray(Wq, np.float32)
    Wk = np.asarray(Wk, np.float32)
    Wv = np.asarray(Wv, np.float32)
    Wo = np.asarray(Wo, np.float32)
    in_maps = [make_inputs_for_core(c, xf, Wq, Wk, Wv, Wo)
               for c in range(N_CORES)]
    res = run_bass_kernel_spmd(nc, in_maps, core_ids=list(range(N_CORES)))
    y = np.concatenate(
        [res.results[c]["y"].astype(np.float32) for c in range(N_CORES)],
        axis=0)
    return y.reshape(1, T, D)
